# revision 1
# baseline (speedup 1.0000x reference)
"""GuidedFilter Bass kernel for TRN2, 8-core batch-parallel.

Problem: B=8, C=3, H=W=1024 fp32, r=20 (causal clamped window-r box filters).
Reference box = cumsum+diff along H then W: out[i] = cs[i] - cs[i-r] (clamped).

Per core: one batch (3 channel images of 1024x1024).
Per channel pipeline (layout: 8 H-tiles of [128 rows, W free]):
  - W-direction box: single tensor_tensor_scan per H-tile over a concatenated
    padded buffer [I | p | I*p | I*I], recurrence state = (x[t]+state)-x[t-r].
  - H-direction box: PE banded matmul (fp32r stage 1, fp16 for A/b stage),
    band weights carry inv_nh(i)/r normalization; PSUM strips [128,512].
  - Stage-2 elementwise (cov/var/A/b) on DVE consuming PSUM strips.
  - Degenerate corner pixel (0,0) (1-sample window, var=0) overwritten
    exactly: A=0, b=p.
"""
import sys
sys.path.insert(0, "/opt/trn_rl_repo")
import numpy as np
from contextlib import ExitStack

import concourse.bass as bass
import concourse.tile as tile
from concourse import bacc, mybir
from concourse.bass_utils import run_bass_kernel_spmd

F32 = mybir.dt.float32
F32R = mybir.dt.float32r
F16 = mybir.dt.float16
ALU = mybir.AluOpType

P = 128          # partitions
H = 1024
W = 1024
C = 3
NB = H // P      # 8 H-blocks
NH = 2           # 512-col halves
EPS = 1e-8

_compiled = None


def _build_consts(r):
    """Band-matmul weights (lhsT layout [K,M]) and column fixup map."""
    inv_nh = np.array([1.0 / min(i + 1, r) for i in range(P)], dtype=np.float64)
    inv_r = 1.0 / r
    # diag block 0: T[m,k] = inv_nh(m)*inv_r  for max(0,m-r+1) <= k <= m
    td0 = np.zeros((P, P), np.float64)
    td = np.zeros((P, P), np.float64)
    tu = np.zeros((P, P), np.float64)
    for m in range(P):
        lo = max(0, m - r + 1)
        td0[m, lo:m + 1] = inv_nh[m] * inv_r
        td[m, lo:m + 1] = inv_r * inv_r
        # subdiag: contributions from previous block rows k in [P-r+1+m, P)
        if m < r - 1:
            tu[m, P - r + 1 + m:P] = inv_r * inv_r
    consts = {
        "wd0": td0.T.astype(np.float32).copy(),
        "wd": td.T.astype(np.float32).copy(),
        "wu": tu.T.astype(np.float32).copy(),
        "wd0h": td0.T.astype(np.float16).copy(),
        "wdh": td.T.astype(np.float16).copy(),
        "wuh": tu.T.astype(np.float16).copy(),
        # per-column fixup for j < r-1: multiply by r/(j+1)
        "colmap": np.tile(
            np.array([r / (j + 1) for j in range(r - 1)], np.float32), (P, 1)
        ).copy(),
    }
    return consts


def _build(r):
    nc = bacc.Bacc("TRN2", target_bir_lowering=False, debug=False, num_devices=8)

    I_d = nc.dram_tensor("I", [C, H, W], F32, kind="ExternalInput").ap()
    p_d = nc.dram_tensor("p", [C, H, W], F32, kind="ExternalInput").ap()
    wd0_d = nc.dram_tensor("wd0", [P, P], F32, kind="ExternalInput").ap()
    wd_d = nc.dram_tensor("wd", [P, P], F32, kind="ExternalInput").ap()
    wu_d = nc.dram_tensor("wu", [P, P], F32, kind="ExternalInput").ap()
    wd0h_d = nc.dram_tensor("wd0h", [P, P], F16, kind="ExternalInput").ap()
    wdh_d = nc.dram_tensor("wdh", [P, P], F16, kind="ExternalInput").ap()
    wuh_d = nc.dram_tensor("wuh", [P, P], F16, kind="ExternalInput").ap()
    colmap_d = nc.dram_tensor("colmap", [P, r - 1], F32, kind="ExternalInput").ap()
    q_d = nc.dram_tensor("q", [C, H, W], F32, kind="ExternalOutput").ap()

    SEG = W + r          # 1044 per-tensor segment (data + r drain zeros)
    BUF4 = r + 4 * SEG   # stage-1 concat buffer free size
    BUF2 = r + 2 * SEG   # A/b concat buffer

    with tile.TileContext(nc) as tc, ExitStack() as ctx:
        wpool = ctx.enter_context(tc.tile_pool(name="weights", bufs=1))
        inbuf = ctx.enter_context(tc.tile_pool(name="inbuf", bufs=3))
        wbox = ctx.enter_context(tc.tile_pool(name="wbox", bufs=3))
        wbox0 = ctx.enter_context(tc.tile_pool(name="wbox0", bufs=1))
        abbuf = ctx.enter_context(tc.tile_pool(name="abbuf", bufs=3))
        wab = ctx.enter_context(tc.tile_pool(name="wab", bufs=3))
        wab0p = ctx.enter_context(tc.tile_pool(name="wab0p", bufs=1))
        s2 = ctx.enter_context(tc.tile_pool(name="s2", bufs=2))
        qpool = ctx.enter_context(tc.tile_pool(name="qpool", bufs=3))
        psum = ctx.enter_context(tc.tile_pool(name="psum", bufs=1, space="PSUM"))
        psum3 = ctx.enter_context(tc.tile_pool(name="psum3", bufs=2, space="PSUM"))

        wd0f = wpool.tile([P, P], F32, tag="wd0f")
        nc.sync.dma_start(wd0f[:], wd0_d[:])
        wd0 = wpool.tile([P, P], F32R, tag="wd0")
        nc.vector.tensor_copy(wd0[:], wd0f[:])
        wdf = wpool.tile([P, P], F32, tag="wdf")
        nc.sync.dma_start(wdf[:], wd_d[:])
        wd = wpool.tile([P, P], F32R, tag="wd")
        nc.vector.tensor_copy(wd[:], wdf[:])
        wuf = wpool.tile([P, P], F32, tag="wuf")
        nc.sync.dma_start(wuf[:], wu_d[:])
        wu = wpool.tile([P, P], F32R, tag="wu")
        nc.vector.tensor_copy(wu[:], wuf[:])
        wd0h = wpool.tile([P, P], F16, tag="wd0h")
        nc.sync.dma_start(wd0h[:], wd0h_d[:])
        wdh = wpool.tile([P, P], F16, tag="wdh")
        nc.sync.dma_start(wdh[:], wdh_d[:])
        wuh = wpool.tile([P, P], F16, tag="wuh")
        nc.sync.dma_start(wuh[:], wuh_d[:])
        colmap = wpool.tile([P, r - 1], F32, tag="colmap")
        nc.sync.dma_start(colmap[:], colmap_d[:])

        for c in range(C):
            wb_tiles = [None] * NB     # stage-1 W-boxed buffers per block
            ab_tiles = [None] * NB     # A/b concat buffers per block
            wab_tiles = [None] * NB    # A/b W-boxed fp16 per block
            in_tiles = [None] * NB     # input concat buffers (I seg reused at q)

            for b in range(NB):
                # ---- stage 1: load, Ip, II, W-box scan ----
                buf = inbuf.tile([P, BUF4], F32, tag="inbuf")
                in_tiles[b] = buf
                # zero the r-wide gaps at positions m*SEG, m=0..3
                gaps = buf[:, 0:4 * SEG].rearrange("p (s q) -> p s q", q=SEG)[:, :, 0:r]
                nc.gpsimd.memset(gaps, 0.0)
                segI = buf[:, r + 0 * SEG:r + 0 * SEG + W]
                segp = buf[:, r + 1 * SEG:r + 1 * SEG + W]
                segIp = buf[:, r + 2 * SEG:r + 2 * SEG + W]
                segII = buf[:, r + 3 * SEG:r + 3 * SEG + W]
                nc.sync.dma_start(segI, I_d[c, b * P:(b + 1) * P, :])
                nc.sync.dma_start(segp, p_d[c, b * P:(b + 1) * P, :])
                nc.vector.tensor_mul(segIp, segI, segp)
                if b == 0:
                    nc.vector.tensor_mul(segII, segI, segI)
                else:
                    nc.scalar.square(segII, segI)

                if b == 0:
                    # block 0 feeds the small-N corner: keep full fp32 for
                    # its own matmuls; F32R copy only for block-1 subdiag.
                    # Separate scans per segment: no drain-residual carry.
                    wb0 = wbox0.tile([P, 4 * SEG], F32, tag="wbox0")
                    for k in range(4):
                        nc.vector.tensor_tensor_scan(
                            wb0[:, k * SEG:k * SEG + W],
                            buf[:, r + k * SEG:r + k * SEG + W],
                            buf[:, k * SEG:k * SEG + W],
                            0.0, ALU.add, ALU.subtract,
                        )
                    wb = wbox.tile([P, 4 * SEG], F32R, tag="wbox")
                    nc.vector.tensor_copy(wb[:], wb0[:])
                else:
                    wb = wbox.tile([P, 4 * SEG], F32R, tag="wbox")
                    for k in range(4):
                        nc.vector.tensor_tensor_scan(
                            wb[:, k * SEG:k * SEG + W],
                            buf[:, r + k * SEG:r + k * SEG + W],
                            buf[:, k * SEG:k * SEG + W],
                            0.0, ALU.add, ALU.subtract,
                        )
                wb_tiles[b] = wb

                # ---- H-box (PE fp32r) + stage 2 per 512-strip ----
                ab = abbuf.tile([P, BUF2], F32, tag="abbuf")
                ab_tiles[b] = ab
                gaps2 = ab[:, 0:2 * SEG].rearrange("p (s q) -> p s q", q=SEG)[:, :, 0:r]
                nc.gpsimd.memset(gaps2, 0.0)
                segA = ab[:, r + 0 * SEG:r + 0 * SEG + W]
                segB = ab[:, r + 1 * SEG:r + 1 * SEG + W]

                for h in range(NH):
                    ps = psum.tile([P, 4 * 512], F32, tag="ps")
                    c0 = h * 512
                    for k in range(4):
                        if b == 0:
                            rhs = wb0[:, k * SEG + c0:k * SEG + c0 + 512]
                            nc.tensor.matmul(
                                ps[:, k * 512:(k + 1) * 512],
                                wd0f[:], rhs,
                                start=True, stop=True,
                            )
                        else:
                            rhs = wb[:, k * SEG + c0:k * SEG + c0 + 512]
                            nc.tensor.matmul(
                                ps[:, k * 512:(k + 1) * 512],
                                wd[:], rhs,
                                start=True, stop=False,
                            )
                    if b > 0:
                        pwb = wb_tiles[b - 1]
                        for k in range(4):
                            rhs = pwb[:, k * SEG + c0:k * SEG + c0 + 512]
                            nc.tensor.matmul(
                                ps[:, k * 512:(k + 1) * 512],
                                wu[:], rhs,
                                start=False, stop=True,
                            )
                    mI = ps[:, 0:512]
                    mp = ps[:, 512:1024]
                    mIp = ps[:, 1024:1536]
                    mII = ps[:, 1536:2048]
                    if h == 0:
                        # left-edge per-column fixup on psum in place (batched)
                        v = ps[:, 0:4 * 512].rearrange(
                            "p (s q) -> p s q", q=512)[:, :, 0:r - 1]
                        cm = colmap[:].unsqueeze(1).broadcast_to([P, 4, r - 1])
                        nc.vector.tensor_mul(v, v, cm)
                    mI_s = s2.tile([P, 512], F32, tag="mI_s")
                    nc.scalar.copy(mI_s[:], mI)
                    mI2 = s2.tile([P, 512], F32, tag="mI2")
                    if b == 0:
                        nc.vector.tensor_mul(mI2[:], mI_s[:], mI_s[:])
                    else:
                        nc.scalar.square(mI2[:], mI)
                    varq = s2.tile([P, 512], F32, tag="varq")
                    nc.vector.scalar_tensor_tensor(
                        varq[:], mII, EPS, mI2[:], ALU.add, ALU.subtract
                    )
                    rv = s2.tile([P, 512], F32, tag="rv")
                    nc.vector.reciprocal_approx_fast(rv[:], varq[:])
                    w_t = s2.tile([P, 512], F32, tag="w_t")
                    nc.vector.tensor_mul(w_t[:], mI_s[:], mp)
                    cov = s2.tile([P, 512], F32, tag="cov")
                    nc.vector.tensor_sub(cov[:], mIp, w_t[:])
                    sA = segA[:, c0:c0 + 512]
                    nc.vector.tensor_mul(sA, cov[:], rv[:])
                    t_t = s2.tile([P, 512], F32, tag="t_t")
                    nc.vector.tensor_mul(t_t[:], sA, mI_s[:])
                    sB = segB[:, c0:c0 + 512]
                    nc.vector.tensor_sub(sB, mp, t_t[:])

                if b == 0:
                    # exact corner: A(0,0)=0, b(0,0)=p(0,0)
                    nc.vector.memset(segA[0:1, 0:1], 0.0)
                    nc.vector.tensor_copy(segB[0:1, 0:1], segp[0:1, 0:1])

                # ---- W-box of A,b (fp16 out; block 0 fp32 for corner) ----
                if b == 0:
                    wab0 = wab0p.tile([P, 2 * SEG], F32, tag="wab0")
                    for k in range(2):
                        nc.vector.tensor_tensor_scan(
                            wab0[:, k * SEG:k * SEG + W],
                            ab[:, r + k * SEG:r + k * SEG + W],
                            ab[:, k * SEG:k * SEG + W],
                            0.0, ALU.add, ALU.subtract,
                        )
                    wabt = wab.tile([P, 2 * SEG], F16, tag="wab")
                    nc.vector.tensor_copy(wabt[:], wab0[:])
                else:
                    wabt = wab.tile([P, 2 * SEG], F16, tag="wab")
                    for k in range(2):
                        nc.vector.tensor_tensor_scan(
                            wabt[:, k * SEG:k * SEG + W],
                            ab[:, r + k * SEG:r + k * SEG + W],
                            ab[:, k * SEG:k * SEG + W],
                            0.0, ALU.add, ALU.subtract,
                        )
                wab_tiles[b] = wabt

                # ---- stage 3 H-box (PE fp16) + q ----
                for h in range(NH):
                    ps3 = psum3.tile([P, 2 * 512], F32, tag="ps3")
                    c0 = h * 512
                    for k in range(2):
                        if b == 0:
                            rhs = wab0[:, k * SEG + c0:k * SEG + c0 + 512]
                            nc.tensor.matmul(
                                ps3[:, k * 512:(k + 1) * 512],
                                wd0f[:], rhs,
                                start=True, stop=True,
                            )
                        else:
                            rhs = wabt[:, k * SEG + c0:k * SEG + c0 + 512]
                            nc.tensor.matmul(
                                ps3[:, k * 512:(k + 1) * 512],
                                wdh[:], rhs,
                                start=True, stop=False,
                            )
                    if b > 0:
                        pwab = wab_tiles[b - 1]
                        for k in range(2):
                            rhs = pwab[:, k * SEG + c0:k * SEG + c0 + 512]
                            nc.tensor.matmul(
                                ps3[:, k * 512:(k + 1) * 512],
                                wuh[:], rhs,
                                start=False, stop=True,
                            )
                    mA = ps3[:, 0:512]
                    mB = ps3[:, 512:1024]
                    if h == 0:
                        v = ps3[:, 0:2 * 512].rearrange(
                            "p (s q) -> p s q", q=512)[:, :, 0:r - 1]
                        cm = colmap[:].unsqueeze(1).broadcast_to([P, 2, r - 1])
                        nc.vector.tensor_mul(v, v, cm)
                    qt = qpool.tile([P, 512], F32, tag="qt")
                    segI_b = in_tiles[b][:, r:r + W]
                    nc.vector.tensor_mul(qt[:], mA, segI_b[:, c0:c0 + 512])
                    nc.vector.tensor_add(qt[:], qt[:], mB)
                    nc.sync.dma_start(
                        q_d[c, b * P:(b + 1) * P, c0:c0 + 512], qt[:]
                    )

    nc.compile()
    return nc


def kernel(I, p, r):
    """Full inputs [8,3,1024,1024] fp32 -> full output, batch-sharded 8 ways."""
    global _compiled
    I = np.ascontiguousarray(np.asarray(I, dtype=np.float32))
    p = np.ascontiguousarray(np.asarray(p, dtype=np.float32))
    r = int(np.asarray(r))
    if _compiled is None or _compiled[0] != r:
        _compiled = (r, _build(r), _build_consts(r))
    _, nc, consts = _compiled
    in_maps = [
        {"I": I[b], "p": p[b], **consts}
        for b in range(8)
    ]
    res = run_bass_kernel_spmd(nc, in_maps, core_ids=list(range(8))).results
    return np.stack([res[b]["q"] for b in range(8)], axis=0)

